# revision 29
# baseline (speedup 1.0000x reference)
"""Expert-parallel MoE routing kernel for Trainium2 (8 NeuronCores).

Problem: out[t] = x[t] @ W[idx[t]].T + b[idx[t]],  idx = pointer_addresses % 8
  x: [2048, 512] f32, W: [8, 8192, 512] f32, b: [8, 8192] f32 -> out [2048, 8192] f32

Strategy: expert parallel. Host computes idx, gathers each expert's tokens
(padded to a common capacity `cap`), and each core e computes
  out_e = x_e @ W[e].T + b[e]
with the vocab dimension on PSUM partitions so the bias is a fused
per-partition bias on the Scalar/Vector engines. Host scatters rows back.

Per-core matmul orientation (out = lhsT.T @ rhs):
  lhsT = W chunk  [K=128 (d inner), M=128 (vocab cols)]   (stationary)
  rhs  = xT chunk [K=128 (d inner), N=cap (tokens)]        (moving)
  psum [128 vocab, cap tokens] accumulated over 4 K-chunks of D=512.

The 64 vocab chunks are streamed in groups; group sizes are graduated
(small first/last) so the serial prologue (first W load) and epilogue
(last out store) are short while steady-state DMAs stay large.
"""

import os

import numpy as np

E = 8          # experts == cores
D = 512        # hidden
V = 8192       # out features
P = 128        # partitions
KCH = D // P   # 4 contraction chunks
VCH = V // P   # 64 vocab chunks

# matmul input dtype: 'f32' (exact, 4 cyc/row), 'f32r' (~full speed, ~1.3e-4
# rel err), 'fp16'/'bf16' (full speed, ~2.6e-4 / ~2.1e-3 rel err)
MM_DTYPE = os.environ.get("KERNEL_MM_DTYPE", "fp16")
# output storage dtype: 'f32' (exact) | 'fp16' (~2.4e-4 quant err, halves out bytes)
OUT_DTYPE = os.environ.get("KERNEL_OUT_DTYPE", "fp16")
_GROUPS_ENV = os.environ.get("KERNEL_GROUPS")
# microbench variants: 'full' | 'dmaonly' (DMAs, no compute) | 'computeonly'
# (compute from one resident W buffer, no steady-state DMA) | 'wonly'
# (W-load DMAs only) | 'mmonly' (matmuls only, no evictions/out)
VARIANT = os.environ.get("KERNEL_VARIANT", "full")
# interleave the two psum-slot accumulation groups' matmuls (A.k0 B.k0 A.k1
# ...) to probe LDWEIGHTS pull-ahead behavior
INTERLEAVE_MM = os.environ.get("KERNEL_INTERLEAVE_MM", "0") == "1"
# which HWDGE ring carries the out stores: 'sync' rides behind the W loads
# (ring-FIFO gives W strict priority), 'scalar' uses the ACT ring
OUT_RING = os.environ.get("KERNEL_OUT_RING", "sync")
# vocab chunks per PSUM tile (2 or 4): more chunks = fewer eviction ops and
# sync edges, but shallower PSUM rotation (8 banks total)
PSUM_CH = int(os.environ.get("KERNEL_PSUM_CH", "2"))
# eviction engine split: 'dve' = all on DVE; 'mix' = alternate tiles between
# DVE (multi-chunk op) and ScalarE (per-chunk activations)
EVICT_MODE = os.environ.get("KERNEL_EVICT", "dve")

LAST_RESULT = None  # BassKernelResults of the most recent run (for test harness)

_BUILD_CACHE = {}


def _in_sz():
    return 2 if MM_DTYPE in ("bf16", "fp16") else 4


def _out_sz():
    return 2 if OUT_DTYPE in ("bf16", "fp16") else 4


def _base_gv(cap):
    """Steady-state vocab chunks per DMA group: as large as SBUF allows.

    Per-partition slab budgets: w tiles gv*KCH*P*in_sz (x3 bufs), o tiles
    gv*cap*out_sz (x2 bufs), plus the resident x tile. 16 fits at the
    nominal cap (~274); shrink for pathologically imbalanced routing.
    """
    for gv in (16, 8, 4, 2, 1):
        if (
            gv * KCH * P * _in_sz() * 3
            + gv * cap * _out_sz() * 2
            + KCH * cap * _in_sz()
            <= 168 * 1024
        ):
            return gv
    return 1


def _groups(cap):
    """Graduated group schedule over the 64 vocab chunks.

    Small first groups so compute starts as soon as ~2 chunks of W land
    (instead of waiting for a full-size load); small last group so the
    serial epilogue (last out store after last compute) is short. Steady
    state uses full-size groups for DMA efficiency.
    """
    if _GROUPS_ENV:
        sched = [int(v) for v in _GROUPS_ENV.split(",")]
        assert sum(sched) == VCH
        return sched
    base = _base_gv(cap)
    if base == 16:
        sched = [2, 4, 8, 16, 16, 10, 6, 2]
    else:
        sched = []
        left = VCH
        while left > 0:
            g = min(base, left)
            sched.append(g)
            left -= g
    assert sum(sched) == VCH
    return sched


def _build(cap, repeat=1, loop_n=1):
    """Build the per-core Bass module for token capacity `cap`.

    repeat/loop_n > 1 re-run the compute loop (same outputs) so the test
    harness can difference wall-times to isolate on-device kernel time;
    loop_n uses a hardware For_i loop (constant code size).
    """
    key = (
        cap, MM_DTYPE, OUT_DTYPE, repeat, loop_n,
        VARIANT, INTERLEAVE_MM, OUT_RING, PSUM_CH,
    )
    if key in _BUILD_CACHE:
        return _BUILD_CACHE[key]

    import concourse.mybir as mybir
    from concourse import bacc
    from concourse.tile import TileContext

    dt_in = {
        "f32": mybir.dt.float32,
        "f32r": mybir.dt.float32r,
        "bf16": mybir.dt.bfloat16,
        "fp16": mybir.dt.float16,
    }[MM_DTYPE]
    f32 = mybir.dt.float32
    out_dt = {"f32": f32, "fp16": mybir.dt.float16, "bf16": mybir.dt.bfloat16}[
        OUT_DTYPE
    ]
    assert cap <= 512, f"psum slot scheme needs cap<=512, got {cap}"

    nc = bacc.Bacc(None, target_bir_lowering=False)
    # flat layouts, vocab-chunk (vi) as the per-partition-contiguous axis
    wt = nc.dram_tensor("wt", [P, VCH, KCH, P], dt_in, kind="ExternalInput")
    xt = nc.dram_tensor("xt", [P, KCH, cap], dt_in, kind="ExternalInput")
    bias = nc.dram_tensor("bias", [P, VCH], f32, kind="ExternalInput")
    out = nc.dram_tensor("out", [P, VCH, cap], out_dt, kind="ExternalOutput")

    # W-load completion granularity (chunks per dma_start): small first
    # groups so compute starts early; all ride the SP HWDGE ring in order.
    W_SCHED = [1, 2, 4] + [8] * 7 + [1]
    assert sum(W_SCHED) == VCH
    # out-store granularity; small last groups shrink the serial epilogue
    OUT_SCHED = [8] * 7 + [4, 2, 1, 1]
    assert sum(OUT_SCHED) == VCH
    assert PSUM_CH in (2, 4) and VCH % PSUM_CH == 0

    do_w_dma = VARIANT in ("full", "dmaonly", "wonly")
    do_compute = VARIANT in ("full", "computeonly", "mmonly")
    do_evict = VARIANT in ("full", "computeonly")
    do_out = VARIANT in ("full", "dmaonly")

    def group_body():
        if VARIANT == "noop":
            return
        if VARIANT.startswith("wbig"):
            g = VARIANT.endswith("g")
            if VARIANT.startswith("wbig1"):
                (nc.gpsimd if g else nc.sync).dma_start(w_all, wt.ap())
            else:
                h = VCH // 2
                (nc.gpsimd if g else nc.sync).dma_start(
                    w_all[:, :h], wt.ap()[:, :h]
                )
                (nc.gpsimd if g else nc.scalar).dma_start(
                    w_all[:, h:], wt.ap()[:, h:]
                )
            return
        # issue the whole W-load stream up front; the permanent buffer has
        # no rotation hazards, and subtile deps release compute per group
        if do_w_dma:
            vi0 = 0
            for nvi in W_SCHED:
                nc.sync.dma_start(
                    w_all[:, vi0 : vi0 + nvi], wt.ap()[:, vi0 : vi0 + nvi]
                )
                vi0 += nvi
        elif do_compute:
            # compute-only benches run from the first W group alone
            nc.sync.dma_start(w_all[:, : W_SCHED[0]], wt.ap()[:, : W_SCHED[0]])
        if do_compute:
            for vi in range(0, VCH, PSUM_CH):
                # multi-chunk PSUM tile: each 512-col f32 slot is exactly one
                # 2KB bank, so matmul writes stay bank-local
                ps = pp.tile([P, PSUM_CH, 512], f32, tag="ps")
                order = (
                    [(c, k) for k in range(KCH) for c in range(PSUM_CH)]
                    if INTERLEAVE_MM
                    else [(c, k) for c in range(PSUM_CH) for k in range(KCH)]
                )
                for c, k in order:
                    wv = vi + c if do_w_dma else (vi + c) % W_SCHED[0]
                    nc.tensor.matmul(
                        ps[:, c, :cap],
                        lhsT=w_all[:, wv, k],
                        rhs=x_sb[:, k],
                        start=(k == 0),
                        stop=(k == KCH - 1),
                    )
                if do_evict:
                    if EVICT_MODE == "single":
                        # evict each slot as soon as its accumulation stops
                        for c in range(PSUM_CH):
                            nc.vector.tensor_tensor(
                                o_all[:, vi + c],
                                ps[:, c, :cap],
                                b_sb[:, vi + c : vi + c + 1].to_broadcast(
                                    (P, cap)
                                ),
                                mybir.AluOpType.add,
                            )
                    elif EVICT_MODE == "mix" and (vi // PSUM_CH) % 2 == 1:
                        for c in range(PSUM_CH):
                            nc.scalar.activation(
                                o_all[:, vi + c],
                                ps[:, c, :cap],
                                mybir.ActivationFunctionType.Identity,
                                bias=b_sb[:, vi + c : vi + c + 1],
                                scale=1.0,
                            )
                    else:
                        # out = psum + bias for all chunks in one DVE op
                        nc.vector.tensor_tensor(
                            o_all[:, vi : vi + PSUM_CH],
                            ps[:, :, :cap],
                            b_sb[:, vi : vi + PSUM_CH].to_broadcast(
                                (P, PSUM_CH, cap)
                            ),
                            mybir.AluOpType.add,
                        )
        if do_out:
            out_eng = nc.sync if OUT_RING == "sync" else nc.scalar
            vi0 = 0
            for nvi in OUT_SCHED:
                out_eng.dma_start(
                    out.ap()[:, vi0 : vi0 + nvi], o_all[:, vi0 : vi0 + nvi]
                )
                vi0 += nvi

    with TileContext(nc) as tc:
        with (
            tc.tile_pool(name="perm", bufs=1) as perm,
            tc.tile_pool(name="pp", bufs=8 // PSUM_CH, space="PSUM") as pp,
        ):
            # x/bias ride the ACT HWDGE ring (fast first-byte, parallel with
            # the first W group on the SP ring) so compute can start early
            x_sb = perm.tile([P, KCH, cap], dt_in, name="x_sb")
            nc.scalar.dma_start(x_sb, xt.ap())
            b_sb = perm.tile([P, VCH], f32, name="b_sb")
            nc.scalar.dma_start(b_sb, bias.ap())
            w_all = perm.tile([P, VCH, KCH, P], dt_in, name="w_all")
            o_all = None
            if do_evict or do_out:
                o_all = perm.tile([P, VCH, cap], out_dt, name="o_all")
            if VARIANT == "dmaonly":
                nc.vector.memset(o_all, 0.0)

            import contextlib

            loop_cm = (
                tc.For_i(0, loop_n, 1) if loop_n > 1 else contextlib.nullcontext()
            )
            with loop_cm:
                for _rep in range(repeat):
                    group_body()

    nc.finalize()
    _BUILD_CACHE[key] = nc
    return nc


def _prepare(x, pointer_addresses, W, b):
    """Host-side shard: gather tokens per expert, lay out per-core inputs."""
    x = np.ascontiguousarray(np.asarray(x), dtype=np.float32)
    W = np.ascontiguousarray(np.asarray(W), dtype=np.float32)
    b = np.ascontiguousarray(np.asarray(b), dtype=np.float32)
    pa = np.asarray(pointer_addresses)

    idx = (pa.astype(np.int64) % E).astype(np.int64)
    rows = [np.flatnonzero(idx == e) for e in range(E)]
    counts = np.array([len(r) for r in rows])
    cap = max(256, int(counts.max()))

    if MM_DTYPE == "bf16":
        import ml_dtypes

        np_dt = np.dtype(ml_dtypes.bfloat16)
    elif MM_DTYPE == "fp16":
        np_dt = np.dtype(np.float16)
    else:
        np_dt = np.dtype(np.float32)

    in_maps = []
    for e in range(E):
        # xT: [P(d inner), KCH, cap]
        x_pad = np.zeros((cap, D), np.float32)
        x_pad[: counts[e]] = x[rows[e]]
        xt_e = np.ascontiguousarray(
            x_pad.reshape(cap, KCH, P).transpose(2, 1, 0).astype(np_dt)
        )
        # wt: [p, vi, k, c] = W[e, vi*P + c, k*P + p]
        w_e = np.ascontiguousarray(
            W[e].reshape(VCH, P, KCH, P).transpose(3, 0, 2, 1).astype(np_dt)
        )
        # bias: [P(c), VCH]
        b_e = np.ascontiguousarray(b[e].reshape(VCH, P).T)
        in_maps.append({"wt": w_e, "xt": xt_e, "bias": b_e})

    return in_maps, rows, counts, cap


def _run(nc, in_maps):
    global LAST_RESULT
    from concourse.bass_utils import run_bass_kernel_spmd

    res = run_bass_kernel_spmd(nc, in_maps, core_ids=list(range(E)))
    LAST_RESULT = res
    return res


def _assemble(res, rows, counts, cap, n_tokens):
    out = np.zeros((n_tokens, V), np.float32)
    for e in range(E):
        # out dram [P(c), VCH, cap] -> vocab-major [V, cap]
        o = (
            res.results[e]["out"]
            .astype(np.float32)
            .transpose(1, 0, 2)
            .reshape(V, cap)
        )
        out[rows[e]] = o[:, : counts[e]].T
    return out


def kernel(x, pointer_addresses, W, b):
    in_maps, rows, counts, cap = _prepare(x, pointer_addresses, W, b)
    nc = _build(cap)
    res = _run(nc, in_maps)
    return _assemble(res, rows, counts, cap, np.asarray(x).shape[0])



# revision 32
# speedup vs baseline: 1.0502x; 1.0502x over previous
"""Expert-parallel MoE routing kernel for Trainium2 (8 NeuronCores).

Problem: out[t] = x[t] @ W[idx[t]].T + b[idx[t]],  idx = pointer_addresses % 8
  x: [2048, 512] f32, W: [8, 8192, 512] f32, b: [8, 8192] f32 -> out [2048, 8192] f32

Strategy: expert parallel. Host computes idx, gathers each expert's tokens
(padded to a common capacity `cap`), and each core e computes
  out_e = x_e @ W[e].T + b[e]
with the vocab dimension on PSUM partitions so the bias is a fused
per-partition add during PSUM eviction. Host scatters rows back.

Per-core matmul orientation (out = lhsT.T @ rhs):
  lhsT = W chunk  [K=128 (d inner), M=128 (vocab cols)]   (stationary)
  rhs  = xT chunk [K=128 (d inner), N=cap (tokens)]        (moving)
  psum [128 vocab, cap tokens] accumulated over 4 K-chunks of D=512.

Measured poles on HW (per core, cap=274): DMA path tops at ~300 GB/s
regardless of engine/ring/transfer size, so the 12.8 MB of traffic (8 MB W
+ 4.5 MB out + x) floors at ~43 us; the PE LDWEIGHTS+MATMUL pair rate at
N=274 is ~154 ns (the ~40 ns/MM weight-load tax over the 114 ns stream is
not hidden by the reorder window), so 256 MMs floor at ~39 us. The kernel
sits at the composition of the two (~45-46 us single-shot).

Layout choices that got there:
- W (64 KB/partition) and the out staging (35 KB/partition) live in
  permanent SBUF buffers -- no pool rotation hazards; subtile deps release
  compute per W dma_start group ([1,2,4,8...] graduated so compute starts
  ~1 us in).
- All W loads AND out stores ride the SP HWDGE ring: ring FIFO drains the
  whole W stream before any out bytes, so compute is never W-starved
  (round-robin between rings would push the last W chunks to ~36 us).
- PSUM tiles hold 2 vocab chunks (one 2 KB bank per 512-col f32 slot);
  each pair is evicted by a single DVE tensor_tensor with a broadcast
  bias add, keeping ScalarE free (ACT's 352-cycle/op overhead made it a
  co-bottleneck when it handled evictions).
"""

import os

import numpy as np

E = 8          # experts == cores
D = 512        # hidden
V = 8192       # out features
P = 128        # partitions
KCH = D // P   # 4 contraction chunks
VCH = V // P   # 64 vocab chunks

# matmul input dtype: 'f32' (exact, 4 cyc/row), 'f32r' (~full speed, ~1.3e-4
# rel err), 'fp16'/'bf16' (full speed, ~2.6e-4 / ~2.1e-3 rel err)
MM_DTYPE = os.environ.get("KERNEL_MM_DTYPE", "fp16")
# output storage dtype: 'f32' (exact) | 'fp16' (~2.4e-4 quant err, halves out bytes)
OUT_DTYPE = os.environ.get("KERNEL_OUT_DTYPE", "fp16")
# microbench variants: 'full' | 'dmaonly' (DMAs, no compute) | 'computeonly'
# (compute from one resident W buffer, no steady-state DMA) | 'wonly'
# (W-load DMAs only) | 'mmonly' (matmuls only, no evictions/out)
VARIANT = os.environ.get("KERNEL_VARIANT", "full")
# interleave the two psum-slot accumulation groups' matmuls (A.k0 B.k0 A.k1
# ...) to probe LDWEIGHTS pull-ahead behavior
INTERLEAVE_MM = os.environ.get("KERNEL_INTERLEAVE_MM", "0") == "1"
# which HWDGE ring carries the out stores: 'sync' rides behind the W loads
# (ring-FIFO gives W strict priority), 'scalar' uses the ACT ring
OUT_RING = os.environ.get("KERNEL_OUT_RING", "sync")
# vocab chunks per PSUM tile (2 or 4): more chunks = fewer eviction ops and
# sync edges, but shallower PSUM rotation (8 banks total)
PSUM_CH = int(os.environ.get("KERNEL_PSUM_CH", "2"))
# eviction engine split: 'dve' = all on DVE; 'mix' = alternate tiles between
# DVE (multi-chunk op) and ScalarE (per-chunk activations)
EVICT_MODE = os.environ.get("KERNEL_EVICT", "dve")

LAST_RESULT = None  # BassKernelResults of the most recent run (for test harness)

_BUILD_CACHE = {}


def _build(cap, repeat=1, loop_n=1):
    """Build the per-core Bass module for token capacity `cap`.

    repeat/loop_n > 1 re-run the compute loop (same outputs) so the test
    harness can difference wall-times to isolate on-device kernel time;
    loop_n uses a hardware For_i loop (constant code size).
    """
    key = (
        cap, MM_DTYPE, OUT_DTYPE, repeat, loop_n,
        VARIANT, INTERLEAVE_MM, OUT_RING, PSUM_CH,
    )
    if key in _BUILD_CACHE:
        return _BUILD_CACHE[key]

    import concourse.mybir as mybir
    from concourse import bacc
    from concourse.tile import TileContext

    dt_in = {
        "f32": mybir.dt.float32,
        "f32r": mybir.dt.float32r,
        "bf16": mybir.dt.bfloat16,
        "fp16": mybir.dt.float16,
    }[MM_DTYPE]
    f32 = mybir.dt.float32
    out_dt = {"f32": f32, "fp16": mybir.dt.float16, "bf16": mybir.dt.bfloat16}[
        OUT_DTYPE
    ]
    assert cap <= 512, f"psum slot scheme needs cap<=512, got {cap}"

    nc = bacc.Bacc(None, target_bir_lowering=False)
    # flat layouts, vocab-chunk (vi) as the per-partition-contiguous axis
    wt = nc.dram_tensor("wt", [P, VCH, KCH, P], dt_in, kind="ExternalInput")
    xt = nc.dram_tensor("xt", [P, KCH, cap], dt_in, kind="ExternalInput")
    bias = nc.dram_tensor("bias", [P, VCH], f32, kind="ExternalInput")
    out = nc.dram_tensor("out", [P, VCH, cap], out_dt, kind="ExternalOutput")

    # W-load completion granularity (chunks per dma_start): small first
    # groups so compute starts early; all ride the SP HWDGE ring in order.
    W_SCHED = [1, 2, 4] + [8] * 7 + [1]
    assert sum(W_SCHED) == VCH
    # out-store granularity; small last groups shrink the serial epilogue
    OUT_SCHED = [8] * 7 + [4, 2, 1, 1]
    assert sum(OUT_SCHED) == VCH
    assert PSUM_CH in (2, 4) and VCH % PSUM_CH == 0

    do_w_dma = VARIANT in ("full", "dmaonly", "wonly")
    do_compute = VARIANT in ("full", "computeonly", "mmonly")
    do_evict = VARIANT in ("full", "computeonly")
    do_out = VARIANT in ("full", "dmaonly")

    def group_body():
        if VARIANT == "noop":
            return
        if VARIANT.startswith("wbig"):
            g = VARIANT.endswith("g")
            if VARIANT.startswith("wbig1"):
                (nc.gpsimd if g else nc.sync).dma_start(w_all, wt.ap())
            else:
                h = VCH // 2
                (nc.gpsimd if g else nc.sync).dma_start(
                    w_all[:, :h], wt.ap()[:, :h]
                )
                (nc.gpsimd if g else nc.scalar).dma_start(
                    w_all[:, h:], wt.ap()[:, h:]
                )
            return
        # issue the whole W-load stream up front; the permanent buffer has
        # no rotation hazards, and subtile deps release compute per group
        if do_w_dma:
            vi0 = 0
            for nvi in W_SCHED:
                nc.sync.dma_start(
                    w_all[:, vi0 : vi0 + nvi], wt.ap()[:, vi0 : vi0 + nvi]
                )
                vi0 += nvi
        elif do_compute:
            # compute-only benches run from the first W group alone
            nc.sync.dma_start(w_all[:, : W_SCHED[0]], wt.ap()[:, : W_SCHED[0]])
        if do_compute:
            for vi in range(0, VCH, PSUM_CH):
                # multi-chunk PSUM tile: each 512-col f32 slot is exactly one
                # 2KB bank, so matmul writes stay bank-local
                ps = pp.tile([P, PSUM_CH, 512], f32, tag="ps")
                order = (
                    [(c, k) for k in range(KCH) for c in range(PSUM_CH)]
                    if INTERLEAVE_MM
                    else [(c, k) for c in range(PSUM_CH) for k in range(KCH)]
                )
                for c, k in order:
                    wv = vi + c if do_w_dma else (vi + c) % W_SCHED[0]
                    nc.tensor.matmul(
                        ps[:, c, :cap],
                        lhsT=w_all[:, wv, k],
                        rhs=x_sb[:, k],
                        start=(k == 0),
                        stop=(k == KCH - 1),
                    )
                if do_evict:
                    if EVICT_MODE == "single":
                        # evict each slot as soon as its accumulation stops
                        for c in range(PSUM_CH):
                            nc.vector.tensor_tensor(
                                o_all[:, vi + c],
                                ps[:, c, :cap],
                                b_sb[:, vi + c : vi + c + 1].to_broadcast(
                                    (P, cap)
                                ),
                                mybir.AluOpType.add,
                            )
                    elif EVICT_MODE == "mix" and (vi // PSUM_CH) % 2 == 1:
                        for c in range(PSUM_CH):
                            nc.scalar.activation(
                                o_all[:, vi + c],
                                ps[:, c, :cap],
                                mybir.ActivationFunctionType.Identity,
                                bias=b_sb[:, vi + c : vi + c + 1],
                                scale=1.0,
                            )
                    else:
                        # out = psum + bias for all chunks in one DVE op
                        nc.vector.tensor_tensor(
                            o_all[:, vi : vi + PSUM_CH],
                            ps[:, :, :cap],
                            b_sb[:, vi : vi + PSUM_CH].to_broadcast(
                                (P, PSUM_CH, cap)
                            ),
                            mybir.AluOpType.add,
                        )
        if do_out:
            out_eng = nc.sync if OUT_RING == "sync" else nc.scalar
            vi0 = 0
            for nvi in OUT_SCHED:
                out_eng.dma_start(
                    out.ap()[:, vi0 : vi0 + nvi], o_all[:, vi0 : vi0 + nvi]
                )
                vi0 += nvi

    with TileContext(nc) as tc:
        with (
            tc.tile_pool(name="perm", bufs=1) as perm,
            tc.tile_pool(name="pp", bufs=8 // PSUM_CH, space="PSUM") as pp,
        ):
            # x/bias ride the ACT HWDGE ring (fast first-byte, parallel with
            # the first W group on the SP ring) so compute can start early
            x_sb = perm.tile([P, KCH, cap], dt_in, name="x_sb")
            nc.scalar.dma_start(x_sb, xt.ap())
            b_sb = perm.tile([P, VCH], f32, name="b_sb")
            nc.scalar.dma_start(b_sb, bias.ap())
            w_all = perm.tile([P, VCH, KCH, P], dt_in, name="w_all")
            o_all = None
            if do_evict or do_out:
                o_all = perm.tile([P, VCH, cap], out_dt, name="o_all")
            if VARIANT == "dmaonly":
                nc.vector.memset(o_all, 0.0)

            import contextlib

            loop_cm = (
                tc.For_i(0, loop_n, 1) if loop_n > 1 else contextlib.nullcontext()
            )
            with loop_cm:
                for _rep in range(repeat):
                    group_body()

    nc.finalize()
    _BUILD_CACHE[key] = nc
    return nc


def _prepare(x, pointer_addresses, W, b):
    """Host-side shard: gather tokens per expert, lay out per-core inputs."""
    x = np.ascontiguousarray(np.asarray(x), dtype=np.float32)
    W = np.ascontiguousarray(np.asarray(W), dtype=np.float32)
    b = np.ascontiguousarray(np.asarray(b), dtype=np.float32)
    pa = np.asarray(pointer_addresses)

    idx = (pa.astype(np.int64) % E).astype(np.int64)
    rows = [np.flatnonzero(idx == e) for e in range(E)]
    counts = np.array([len(r) for r in rows])
    cap = max(256, int(counts.max()))

    if MM_DTYPE == "bf16":
        import ml_dtypes

        np_dt = np.dtype(ml_dtypes.bfloat16)
    elif MM_DTYPE == "fp16":
        np_dt = np.dtype(np.float16)
    else:
        np_dt = np.dtype(np.float32)

    in_maps = []
    for e in range(E):
        # xT: [P(d inner), KCH, cap]
        x_pad = np.zeros((cap, D), np.float32)
        x_pad[: counts[e]] = x[rows[e]]
        xt_e = np.ascontiguousarray(
            x_pad.reshape(cap, KCH, P).transpose(2, 1, 0).astype(np_dt)
        )
        # wt: [p, vi, k, c] = W[e, vi*P + c, k*P + p]
        w_e = np.ascontiguousarray(
            W[e].reshape(VCH, P, KCH, P).transpose(3, 0, 2, 1).astype(np_dt)
        )
        # bias: [P(c), VCH]
        b_e = np.ascontiguousarray(b[e].reshape(VCH, P).T)
        in_maps.append({"wt": w_e, "xt": xt_e, "bias": b_e})

    return in_maps, rows, counts, cap


def _run(nc, in_maps):
    global LAST_RESULT
    from concourse.bass_utils import run_bass_kernel_spmd

    res = run_bass_kernel_spmd(nc, in_maps, core_ids=list(range(E)))
    LAST_RESULT = res
    return res


def _assemble(res, rows, counts, cap, n_tokens):
    out = np.zeros((n_tokens, V), np.float32)
    for e in range(E):
        # out dram [P(c), VCH, cap] -> vocab-major [V, cap]
        o = (
            res.results[e]["out"]
            .astype(np.float32)
            .transpose(1, 0, 2)
            .reshape(V, cap)
        )
        out[rows[e]] = o[:, : counts[e]].T
    return out


def kernel(x, pointer_addresses, W, b):
    in_maps, rows, counts, cap = _prepare(x, pointer_addresses, W, b)
    nc = _build(cap)
    res = _run(nc, in_maps)
    return _assemble(res, rows, counts, cap, np.asarray(x).shape[0])

